# revision 11
# baseline (speedup 1.0000x reference)
"""GATv2 attention head (kgcnn AttentionHeadGATV2) on 8 Trainium2 NeuronCores.

Strategy (edge/graph parallelism, self-contained — no file reads):
  * Host: bucket edges by receiver node; core c owns receiver nodes
    [c*6250, (c+1)*6250). Within a core, edges are grouped into 49
    receiver "windows" of 128 nodes. Slots are split into a lo region
    (send < 32768) and a hi region so gather indices fit int16; counts
    are static per window (max over the 8 cores, padded to 16/128).
  * Device precompute (replicated per core): psw = [P_s | w_n] with
    P_s = node @ (W_att[128:]*c)[:,perm], w_n = node @ W_lin computed
    with a single fused 256-col matmul per 128-row chunk; the node is
    pre-transposed on the host (no DMA transposes). A per-core shard
    of P_r = node @ (W_att[:128]*c)[:,perm] + b_hat. The column scaling
    c and permutation fold a_vec into the leaky-relu (sign trick), so
    the attention logit is a plain row-sum. PSUM->SBUF copies alternate
    between the Scalar and Vector engines. psw is written as separate
    lo/hi tables so lo-half gathers can start earlier.
  * Device edge phase, per window: one merged dma_gather per table
    (pr / psw-lo / psw-hi) with per-window exact descriptor counts;
    z = pr + ps; h = lrelu_{0.2/5}(z) via (z*a) max/min z on DVE
    (column split at k); a = rowsum(h) via per-subtile tensor_scalar
    accumulate; ex = exp(a) on the Scalar engine; one-hot (x ex) built
    by fused tensor_scalar(is_equal, mult); scatter-add via PE matmul
    into PSUM [128 nodes x 128] (+denominator column);
    out = lrelu_{0.2}(numer/denom + b_lin).
"""

import sys

sys.path.insert(0, "/opt/trn_rl_repo")

import numpy as np
import ml_dtypes

import concourse.bacc as bacc
import concourse.bass as bass
import concourse.mybir as mybir
import concourse.tile as tile
from concourse import bass_utils

DT = mybir.dt
ALU = mybir.AluOpType
ACTF = mybir.ActivationFunctionType

BF16 = ml_dtypes.bfloat16

# Problem constants (hardcoded per the task contract).
N_NODES = 50000
N_EDGES = 800000
F_DIM = 128
UNITS = 128
ALPHA = 0.2
NCORES = 8
NPC = N_NODES // NCORES          # 6250 receiver nodes per core
WIN = 128                        # receiver-window size (PSUM partition dim)
NW = (NPC + WIN - 1) // WIN      # 49 windows per core
LAST_ROWS = NPC - (NW - 1) * WIN # 106 rows in the last window
ST_ROWS = 2048                   # precompute supertile rows
STJ = ST_ROWS // 128             # 16 chunks per supertile
NODE_PAD = 51200                 # 25 * 2048
LO_ROWS = 32768                  # 16 supertiles -> psw_lo table
HI_ROWS = NODE_PAD - LO_ROWS     # 9 supertiles -> psw_hi table
SHARD_PAD = 8192                 # 4 * 2048
LO_BASE = 32768                  # int16 gather-index split point
PRE = 3                          # pr-gather lookahead windows

_BUILD_CACHE = {}


def _ru(x, m):
    return (int(x) + m - 1) // m * m


def _build(sizes, k_pos, compile=True):
    """sizes = (PL, nhi) per window: lo-region receiver-pair count and
    hi-region slot count, both 128-rounded max-over-core values."""
    PL = [s[0] for s in sizes]
    nhi = [s[1] for s in sizes]
    nw = len(sizes)
    nlo = [2 * p for p in PL]
    ntot = [nlo[w] + nhi[w] for w in range(nw)]
    nsub = [(ntot[w] + 127) // 128 for w in range(nw)]
    nsub_max = max(nsub)
    gcol = [PL[w] // 128 + nhi[w] // 128 for w in range(nw)]
    gcol_max = max(gcol)
    ri_off = np.cumsum([0] + [(PL[w] + nhi[w]) // 16 for w in range(nw)])
    sl_off = np.cumsum([0] + [n // 16 for n in nlo])
    sh_off = np.cumsum([0] + [n // 16 for n in nhi])
    ra_off = np.cumsum([0] + nsub)
    RI, SL, SH, RA = int(ri_off[-1]), int(sl_off[-1]), int(sh_off[-1]), \
        int(ra_off[-1])

    nc = bacc.Bacc("TRN2", target_bir_lowering=False, debug=False,
                   num_devices=NCORES, num_swdge_queues=4)

    node_t_d = nc.dram_tensor("node_t", [128, NODE_PAD], DT.bfloat16,
                              kind="ExternalInput")
    shard_t_d = nc.dram_tensor("shard_t", [128, SHARD_PAD], DT.bfloat16,
                               kind="ExternalInput")
    wsl_d = nc.dram_tensor("wsl", [128, 256], DT.bfloat16,
                           kind="ExternalInput")
    wr_d = nc.dram_tensor("wr", [128, 128], DT.bfloat16, kind="ExternalInput")
    blin_d = nc.dram_tensor("blin", [128, 128], DT.float32,
                            kind="ExternalInput")
    biasr_d = nc.dram_tensor("biasr", [128, 128], DT.float32,
                             kind="ExternalInput")
    iota_d = nc.dram_tensor("iota", [128, 128], DT.bfloat16,
                            kind="ExternalInput")
    recvidx_d = nc.dram_tensor("recvidx", [128, RI], DT.int16,
                               kind="ExternalInput")
    sendlo_d = nc.dram_tensor("sendlo", [128, SL], DT.int16,
                              kind="ExternalInput")
    sendhi_d = nc.dram_tensor("sendhi", [128, SH], DT.int16,
                              kind="ExternalInput")
    recvadj_d = nc.dram_tensor("recvadj", [128, RA], DT.float32,
                               kind="ExternalInput")
    out_d = nc.dram_tensor("out", [NPC, 128], DT.float32,
                           kind="ExternalOutput")

    with tile.TileContext(nc) as tc:
        with (
            tc.tile_pool(name="consts", bufs=1) as cpool,
            tc.tile_pool(name="dram", bufs=1, space="DRAM") as dpool,
        ):
            psw_lo = dpool.tile([LO_ROWS, 256], DT.bfloat16)
            psw_hi = dpool.tile([HI_ROWS, 256], DT.bfloat16)
            pr_dram = dpool.tile([SHARD_PAD, 128], DT.bfloat16)

            wsl_sb = cpool.tile([128, 256], DT.bfloat16)
            nc.sync.dma_start(wsl_sb[:], wsl_d[:])
            wr_sb = cpool.tile([128, 128], DT.bfloat16)
            nc.sync.dma_start(wr_sb[:], wr_d[:])
            blin_sb = cpool.tile([128, 128], DT.float32)
            nc.sync.dma_start(blin_sb[:], blin_d[:])
            biasr_sb = cpool.tile([128, 128], DT.float32)
            nc.sync.dma_start(biasr_sb[:], biasr_d[:])
            iota_sb = cpool.tile([128, 128], DT.bfloat16)
            nc.sync.dma_start(iota_sb[:], iota_d[:])
            recvidx_sb = cpool.tile([128, RI], DT.int16)
            nc.sync.dma_start(recvidx_sb[:], recvidx_d[:])
            sendlo_sb = cpool.tile([128, SL], DT.int16)
            nc.sync.dma_start(sendlo_sb[:], sendlo_d[:])
            sendhi_sb = cpool.tile([128, SH], DT.int16)
            nc.sync.dma_start(sendhi_sb[:], sendhi_d[:])
            recvadj_sb = cpool.tile([128, RA], DT.float32)
            nc.sync.dma_start(recvadj_sb[:], recvadj_d[:])
            ones_sb = cpool.tile([128, 1], DT.bfloat16)
            nc.vector.memset(ones_sb[:], 1.0)

            with tc.tile_pool(name="gprp", bufs=PRE + 2) as gp:

                def gpr_gather(w):
                    t = gp.tile([128, nsub_max, 128], DT.bfloat16, tag="gpr")
                    nc.gpsimd.dma_gather(
                        t[:, 0:nsub[w], :], pr_dram[:],
                        recvidx_sb[:, int(ri_off[w]):int(ri_off[w + 1])],
                        ntot[w], ntot[w], 128, queue_num=0)
                    return t

                gpr_tiles = {}
                with (
                    tc.tile_pool(name="pcsb", bufs=3) as pc,
                    tc.tile_pool(name="pcsb2", bufs=2) as pc2,
                    tc.tile_pool(name="pcpsum", bufs=4, space="PSUM") as pcp,
                ):
                    # ---------------- precompute: shard P_r ----------------
                    for st in range(SHARD_PAD // ST_ROWS):
                        sh = pc.tile([128, ST_ROWS], DT.bfloat16, tag="nT")
                        nc.sync.dma_start(
                            sh[:],
                            shard_t_d[:, st * ST_ROWS:(st + 1) * ST_ROWS])
                        prrows = pc2.tile([128, STJ, 128], DT.bfloat16,
                                          tag="prrow")
                        for j in range(STJ):
                            ps = pcp.tile([128, 256], DT.float32, tag="pcps")
                            nc.tensor.matmul(ps[:, 0:128],
                                             sh[:, j * 128:(j + 1) * 128],
                                             wr_sb[:], start=True, stop=True)
                            nc.vector.tensor_tensor(prrows[:, j, :],
                                                    ps[:, 0:128],
                                                    biasr_sb[:], ALU.add)
                        r0 = st * ST_ROWS
                        nc.sync.dma_start(pr_dram[r0:r0 + ST_ROWS, :],
                                          prrows[:])

                    # ------ prologue: first pr gathers overlap psw ------
                    for w in range(min(PRE, nw)):
                        gpr_tiles[w] = gpr_gather(w)

                    # ---------------- precompute: psw tables ----------------
                    for st in range(NODE_PAD // ST_ROWS):
                        nt = pc.tile([128, ST_ROWS], DT.bfloat16, tag="nT")
                        nc.sync.dma_start(
                            nt[:], node_t_d[:, st * ST_ROWS:(st + 1) * ST_ROWS])
                        rows = pc.tile([128, STJ, 256], DT.bfloat16,
                                       tag="pswrow")
                        for j in range(STJ):
                            ps = pcp.tile([128, 256], DT.float32, tag="pcps")
                            nc.tensor.matmul(ps[:],
                                             nt[:, j * 128:(j + 1) * 128],
                                             wsl_sb[:], start=True, stop=True)
                            if j % 2 == 0:
                                nc.scalar.copy(rows[:, j, :], ps[:])
                            else:
                                nc.vector.tensor_scalar_mul(rows[:, j, :],
                                                            ps[:], 1.0)
                        if st < LO_ROWS // ST_ROWS:
                            r0 = st * ST_ROWS
                            nc.sync.dma_start(psw_lo[r0:r0 + ST_ROWS, :],
                                              rows[:])
                        else:
                            r0 = st * ST_ROWS - LO_ROWS
                            nc.sync.dma_start(psw_hi[r0:r0 + ST_ROWS, :],
                                              rows[:])

                # ---------------- edge phase ----------------
                with (
                    tc.tile_pool(name="edge", bufs=2) as ep,
                    tc.tile_pool(name="edge3", bufs=3) as ep3,
                    tc.tile_pool(name="small", bufs=4) as sp,
                    tc.tile_pool(name="epsum", bufs=3, space="PSUM") as pp,
                ):
                    for w in range(nw):
                    if w + PRE < nw:
                        gpr_tiles[w + PRE] = gpr_gather(w + PRE)
                    gpr = gpr_tiles.pop(w)
                    ns = nsub[w]
                    nls = nlo_pad[w] // 128

                    gpsw = ep3.tile([128, nsub_max, 256], DT.bfloat16,
                                    tag="gpsw")
                    if w < 3:
                        nc.vector.memset(gpsw[:], 0.0)
                    nc.gpsimd.dma_gather(
                        gpsw[:, 0:nls, :], psw_lo[:],
                        sendlo_sb[:, int(sl_off[w]):int(sl_off[w + 1])],
                        nlo[w], nlo[w], 256, queue_num=0)
                    nc.gpsimd.dma_gather(
                        gpsw[:, nls:ns, :], psw_hi[:],
                        sendhi_sb[:, int(sh_off[w]):int(sh_off[w + 1])],
                        nhi[w], nhi[w], 256, queue_num=0)

                    z = ep.tile([128, nsub_max, 128], DT.bfloat16, tag="z")
                    nc.vector.tensor_tensor(z[:, 0:ns, :], gpr[:, 0:ns, :],
                                            gpsw[:, 0:ns, 0:128], ALU.add)
                    h = ep.tile([128, nsub_max, 128], DT.bfloat16, tag="h")
                    if k_pos > 0:
                        nc.vector.scalar_tensor_tensor(
                            h[:, 0:ns, 0:k_pos], z[:, 0:ns, 0:k_pos], ALPHA,
                            z[:, 0:ns, 0:k_pos], ALU.mult, ALU.max)
                    if k_pos < 128:
                        nc.vector.scalar_tensor_tensor(
                            h[:, 0:ns, k_pos:128], z[:, 0:ns, k_pos:128],
                            1.0 / ALPHA, z[:, 0:ns, k_pos:128], ALU.mult,
                            ALU.min)
                    a = sp.tile([128, nsub_max], DT.float32, tag="a")
                    for s in range(ns):
                        nc.vector.tensor_scalar(h[:, s, :], h[:, s, :], 1.0,
                                                None, ALU.mult, ALU.add,
                                                accum_out=a[:, s:s + 1])
                    ex = sp.tile([128, nsub_max], DT.float32, tag="ex")
                    nc.scalar.activation(ex[:, 0:ns], a[:, 0:ns], ACTF.Exp)

                    oh = ep3.tile([128, nsub_max, 128], DT.bfloat16, tag="oh")
                    ra0 = int(ra_off[w])
                    for s in range(ns):
                        nc.vector.tensor_scalar(
                            oh[:, s, :], iota_sb[:],
                            recvadj_sb[:, ra0 + s:ra0 + s + 1],
                            ex[:, s:s + 1], ALU.is_equal, ALU.mult)

                    pw_n = pp.tile([128, 128], DT.float32, tag="pwn")
                    pw_d = pp.tile([128, 1], DT.float32, tag="pwd")
                    for s in range(ns):
                        nc.tensor.matmul(pw_n[:], oh[:, s, :],
                                         gpsw[:, s, 128:256],
                                         start=(s == 0), stop=(s == ns - 1),
                                         skip_group_check=True)
                        nc.tensor.matmul(pw_d[:], oh[:, s, :],
                                         ones_sb[:],
                                         start=(s == 0), stop=(s == ns - 1),
                                         skip_group_check=True)

                    dn = sp.tile([128, 1], DT.float32, tag="dn")
                    nc.vector.tensor_scalar(dn[:], pw_d[:], 1e-30, None,
                                            ALU.add)
                    rn = sp.tile([128, 1], DT.float32, tag="rn")
                    nc.vector.reciprocal(rn[:], dn[:])
                    o1 = sp.tile([128, 128], DT.float32, tag="o1")
                    nc.vector.scalar_tensor_tensor(o1[:], pw_n[:], rn[:],
                                                   blin_sb[:], ALU.mult,
                                                   ALU.add)
                    o2 = sp.tile([128, 128], DT.float32, tag="o2")
                    nc.vector.scalar_tensor_tensor(o2[:], o1[:], ALPHA, o1[:],
                                                   ALU.mult, ALU.max)
                    rows_out = WIN if w < nw - 1 else LAST_ROWS
                    nc.sync.dma_start(out_d[w * WIN:w * WIN + rows_out, :],
                                      o2[0:rows_out, :])

    if compile:
        nc.compile()
    return nc


def _store_perm(i):
    """Map a logical row index to its stored row in psw/pr DRAM.

    The precompute writes SBUF tiles [128, J, C] with one dma_start per
    ST_ROWS block; the DMA lays out (partition p, subtile j) at block row
    p*J + j, while logical row r = j*128 + p. Gather indices must follow."""
    b = i // ST_ROWS
    r = i % ST_ROWS
    return b * ST_ROWS + (r % 128) * STJ + (r // 128)


def _wrap16(arr):
    """[n] int16 slot-index array (n % 16 == 0) -> [128, n//16] wrapped and
    replicated across the 8 GPSIMD core partition groups."""
    n = arr.shape[0]
    w = arr.reshape(n // 16, 16).T
    return np.tile(w, (8, 1))


def _prep(node, edge_index, W_lin, b_lin, W_att, b_att, a_vec):
    """Host-side sharding/preprocessing. Returns (in_maps, sizes, k)."""
    recv = np.asarray(edge_index[0], dtype=np.int64)
    send = np.asarray(edge_index[1], dtype=np.int64)

    # ---- fold a_vec into the weights (sign trick) ----
    sa = np.asarray(a_vec[:, 0], dtype=np.float32)
    pos = sa >= 0
    perm = np.concatenate([np.where(pos)[0], np.where(~pos)[0]])
    k = int(pos.sum())
    cvec = np.where(pos, sa, ALPHA * sa).astype(np.float32)

    W_att = np.asarray(W_att, dtype=np.float32)
    w_r_hat = (W_att[:F_DIM, :] * cvec[None, :])[:, perm]
    w_s_hat = (W_att[F_DIM:, :] * cvec[None, :])[:, perm]
    b_hat = (np.asarray(b_att, np.float32) * cvec)[perm]

    # ---- edge bucketing ----
    cid = recv // NPC
    rloc = recv - cid * NPC
    wid = rloc >> 7
    grp = (send >= LO_BASE).astype(np.int64)
    key = (cid * NW + wid) * 2 + grp
    order = np.lexsort((send, rloc, key))
    rs, ss = rloc[order], send[order]
    counts = np.bincount(key, minlength=NCORES * NW * 2).reshape(NCORES, NW, 2)
    starts = np.concatenate([[0], np.cumsum(counts.reshape(-1))])[:-1].reshape(
        NCORES, NW, 2)

    sp_send = _store_perm(ss)
    sp_recv_all = _store_perm(np.arange(SHARD_PAD))

    def pack_pairs(r):
        """Same-receiver pairing of a sorted rloc array. Returns
        (pair_idx, t, pair_rloc): slot assignments and per-pair receiver."""
        n = len(r)
        if n == 0:
            return (np.zeros(0, np.int64), np.zeros(0, np.int64),
                    np.zeros(0, np.int64))
        new_g = np.r_[True, r[1:] != r[:-1]]
        firsts = np.flatnonzero(new_g)
        deg = np.diff(np.r_[firsts, n])
        occ = np.arange(n) - np.repeat(firsts, deg)
        ppg = (deg + 1) // 2
        pair_base = np.repeat(np.r_[0, np.cumsum(ppg)[:-1]], deg)
        pair_idx = pair_base + occ // 2
        P = int(ppg.sum())
        pair_rloc = np.zeros(P, np.int64)
        pair_rloc[pair_idx] = r
        return pair_idx, occ % 2, pair_rloc

    # first pass: per-(core, window) lo pair counts -> shared sizes
    pairs_cw = {}
    PLc = np.zeros((NCORES, NW), np.int64)
    for c in range(NCORES):
        for w in range(NW):
            b0, clo = starts[c, w, 0], counts[c, w, 0]
            pi, t, prl = pack_pairs(rs[b0:b0 + clo])
            pairs_cw[(c, w)] = (pi, t, prl)
            PLc[c, w] = len(prl)
    PL = [_ru(PLc[:, w].max(), 128) for w in range(NW)]
    nhi = [_ru(max(counts[:, w, 1].max(), 1), 128) for w in range(NW)]
    sizes = tuple(zip(PL, nhi))
    nlo = [2 * p for p in PL]
    ntot = [nlo[w] + nhi[w] for w in range(NW)]
    nsub = [t // 128 for t in ntot]
    nri = [(PL[w] + nhi[w]) for w in range(NW)]

    recvidx_l, sendlo_l, sendhi_l, recvadj_l = [], [], [], []
    for c in range(NCORES):
        ri = np.zeros(sum(nri), np.int16)
        sl = np.zeros(sum(nlo), np.int16)
        sh = np.zeros(sum(nhi), np.int16)
        ra = np.full(sum(nsub) * 128, -1000.0, np.float32)
        ri_base = np.cumsum([0] + nri)
        sl_base = np.cumsum([0] + nlo)
        sh_base = np.cumsum([0] + nhi)
        ra_base = np.cumsum([0] + nsub) * 128
        for w in range(NW):
            clo = counts[c, w, 0]
            b0 = starts[c, w, 0]
            pi, t, prl = pairs_cw[(c, w)]
            # gpr pair indexes (shard store-perm of each pair's receiver)
            ri[ri_base[w]:ri_base[w] + len(prl)] = sp_recv_all[prl]
            # lo slots: pair p' slot t -> partition p'%128, col 2*(p'//128)+t
            col = 2 * (pi // 128) + t
            par = pi % 128
            g = col * 128 + par
            sl[sl_base[w] + g] = sp_send[b0:b0 + clo]
            ra[ra_base[w] + g] = rs[b0:b0 + clo] - w * WIN
            # hi region: plain slots after the lo cols
            chi = counts[c, w, 1]
            b1 = starts[c, w, 1]
            o = ri_base[w] + PL[w]
            ri[o:o + chi] = sp_recv_all[rs[b1:b1 + chi]]
            sh[sh_base[w]:sh_base[w] + chi] = sp_send[b1:b1 + chi] - LO_BASE
            ra[ra_base[w] + nlo[w]:ra_base[w] + nlo[w] + chi] = \
                rs[b1:b1 + chi] - w * WIN
        recvidx_l.append(_wrap16(ri))
        sendlo_l.append(_wrap16(sl))
        sendhi_l.append(_wrap16(sh))
        # recvadj: slot g = s*128 + p lives at [p, col s]
        recvadj_l.append(np.ascontiguousarray(
            ra.reshape(-1, 128).T.astype(np.float32)))

    # ---- node arrays (host pre-transposed) ----
    node = np.asarray(node, dtype=np.float32)
    node_pad = np.zeros((NODE_PAD, F_DIM), np.float32)
    node_pad[:N_NODES] = node
    node_t = np.ascontiguousarray(node_pad.T.astype(BF16))

    iota = np.tile(np.arange(128, dtype=np.float32), (128, 1)).astype(BF16)
    blin = np.tile(np.asarray(b_lin, np.float32)[None, :], (128, 1))
    biasr = np.tile(b_hat[None, :], (128, 1)).astype(np.float32)
    wsl = np.concatenate([w_s_hat, np.asarray(W_lin, np.float32)],
                         axis=1).astype(BF16)

    in_maps = []
    for c in range(NCORES):
        shard = np.zeros((SHARD_PAD, F_DIM), np.float32)
        shard[:NPC] = node[c * NPC:(c + 1) * NPC]
        shard_t = np.ascontiguousarray(shard.T.astype(BF16))
        in_maps.append({
            "node_t": node_t,
            "shard_t": shard_t,
            "wsl": wsl,
            "wr": w_r_hat.astype(BF16),
            "blin": blin,
            "biasr": biasr,
            "iota": iota,
            "recvidx": recvidx_l[c],
            "sendlo": sendlo_l[c],
            "sendhi": sendhi_l[c],
            "recvadj": recvadj_l[c],
        })
    return in_maps, sizes, k


def kernel(node, edge, edge_index, W_lin, b_lin, W_att, b_att, a_vec):
    in_maps, sizes, k = _prep(node, edge_index, W_lin, b_lin,
                              W_att, b_att, a_vec)
    ckey = (sizes, k)
    if ckey not in _BUILD_CACHE:
        _BUILD_CACHE[ckey] = _build(sizes, k)
    nc = _BUILD_CACHE[ckey]
    res = bass_utils.run_bass_kernel_spmd(nc, in_maps,
                                          core_ids=list(range(NCORES)))
    out = np.concatenate([res.results[c]["out"] for c in range(NCORES)],
                         axis=0)
    return np.ascontiguousarray(out[:N_NODES]).astype(np.float32)
